# revision 92
# baseline (speedup 1.0000x reference)
"""DSS layer kernel for Trainium2 (8 NeuronCores, SPMD, no collectives).

Math: the reference's FFT conv kernel k[h,l] = Re(Wc @ exp(Lam*t)) has
|exp(Lam*t)| = e^{-l/2}, so taps beyond m=64 are < 1e-12 relative -- the conv
is a 65-tap causal FIR. We implement it as overlap-save block convolution:
  - window F=256, hop=192, left halo 64
  - half-shifted real DFT (bins f+1/2, f=0..127): exactly 128 complex bins
    (no DC/Nyquist degeneracy), diagonalizes negacyclic convolution; the
    aliased first 64 outputs of each window are discarded, so linear (causal)
    convolution is exact.
  - the kernel spectrum khat (from frequencies/decays/W -- all small
    parameter inputs) is folded on the host in float64 and shipped as bf16,
    so the device only runs: forward DFT (PE), spectrum product (elementwise),
    inverse DFT (PE), gelu (ACT), the 512x512 linear (PE), gelu (ACT), store.
Sharding: 8 cores = (batch b, L-half). Each core computes all 512 channels
for its 1024 time steps, so the final linear needs no cross-core comm.

Schedule (per core, tuned against the TimelineSim cost model):
  - all tensors move as bf16 (u rounded on host; y2 stored as bf16 bits and
    upconverted on host), halving the serialized DMA-pipe traffic; all
    matmuls are bf16 (1 cycle/row, cost = output free size per the model)
  - the spectrum product is fused: ACT casts ur PSUM->bf16, DVE does one
    broadcast-AP wide mult for both ui products and (with Pool taking the
    pi-lane ur product mid-stream) the pr/pi combines; the last two windows
    stay all-DVE so the tail chain is short
  - per-window linear chunks follow two windows behind the conv pipeline,
    keeping the PE busy while each window's spectrum chain runs; the last
    two chunks accumulate in retired forward-DFT PSUM banks so they skip the
    shared-bank WAR chain; stores are staged so only a 64-column store
    remains after the final gelu
  - a dummy gelu preloads the ACT table and ~3us of throwaway matmuls ramp
    the PE p-state while the first DMAs land
"""

import numpy as np

H = 512
N = 64
B = 4
L = 2048
K = 65          # FIR taps
F = 256         # DFT window
HOP = 192
HALO = 64
NWIN = 6
LLOC = L // 2   # 1024 per core
ROWS = HALO + NWIN * HOP   # 1216 rows of u^T per core
RPAD = 1280                # padded to 10 * 128
HT = H // 128   # 4 h-tiles
NCORES = 8
NWARM = 16      # PE warmup matmuls (start the pstate ramp clock early)

_cache = {}


def _build_nc(has_bias=False):
    import concourse.bacc as bacc
    import concourse.tile as tile
    from concourse import mybir

    f32 = mybir.dt.float32
    bf16 = mybir.dt.bfloat16
    u16 = mybir.dt.uint16
    GELU = mybir.ActivationFunctionType.Gelu
    COPY = mybir.ActivationFunctionType.Copy

    nc = bacc.Bacc(None, target_bir_lowering=False)

    ut = nc.dram_tensor("ut", [RPAD, H], u16, kind="ExternalInput")
    dfblob = nc.dram_tensor("dfblob", [128, 512], u16, kind="ExternalInput")
    kblob = nc.dram_tensor("kblob", [128, 1544], u16, kind="ExternalInput")
    lwt = nc.dram_tensor("lwt", [H, H], u16, kind="ExternalInput")
    y2 = nc.dram_tensor("y2", [H, LLOC], u16, kind="ExternalOutput")

    with tile.TileContext(nc) as tc:
        with (
            tc.tile_pool(name="consts", bufs=1) as consts,
            tc.tile_pool(name="scratch", bufs=3) as scratch,
        ):
            # ---------- loads: one queue (SP), in consumption order ----------
            # warmup rhs is 304 wide so the 12 warmups accumulate ~3us of PE
            # busy time, finishing the p-state ramp right as window 0's data
            # lands
            warm_sb = consts.tile([128, 304], bf16, tag="warm")
            nc.vector.memset(warm_sb, 0.0)

            # dfblob layout: [dfc_a0 | dfsn_a0 | dfc_a1 | dfsn_a1]
            dfblob_sb = consts.tile([128, 512], bf16, tag="dfblob")
            nc.sync.dma_start(out=dfblob_sb, in_=dfblob[:, :].bitcast(bf16))
            dfc_sb = [dfblob_sb[:, 0:128], dfblob_sb[:, 256:384]]
            dfsn_sb = [dfblob_sb[:, 128:256], dfblob_sb[:, 384:512]]

            u_sb = consts.tile([128, 8, H], bf16, tag="u_sb")
            u2_sb = consts.tile([128, 9, H], bf16, tag="u2_sb")

            def load_u(which, q, n=2):
                if which == "u":
                    nc.sync.dma_start(
                        out=u_sb[:, q:q + n, :],
                        in_=ut[q * 128:(q + n) * 128, :].bitcast(bf16)
                        .rearrange("(q p) h -> p q h", p=128))
                else:
                    nc.sync.dma_start(
                        out=u2_sb[:, q:q + n, :],
                        in_=ut[64 + q * 128:64 + (q + n) * 128, :].bitcast(bf16)
                        .rearrange("(q p) h -> p q h", p=128))

            load_u("u", 0)                               # window 0
            load_u("u2", 1)                              # window 1
            # kblob: [krb|kib] pair (shared by both fused spectrum mults),
            # then inverse-DFT tables and the bias bits; loaded as two DMAs
            # in need order (spectrum product before inverse tables)
            kblob_sb = consts.tile([128, 1544], bf16, tag="kblob")
            nc.sync.dma_start(out=kblob_sb[:, 0:1024],
                              in_=kblob[:, 0:1024].bitcast(bf16))
            kr_pair = kblob_sb[:, 0:1024].rearrange("p (a f) -> p a f", a=2)
            krb_sb = kblob_sb[:, 0:512]
            kib_sb = kblob_sb[:, 512:1024]
            icc_sb = kblob_sb[:, 1024:1280]
            icsn_sb = kblob_sb[:, 1280:1536]
            lb_sb = kblob_sb[:, 1536:1544].bitcast(f32)
            load_u("u", 3)                               # window 2
            nc.sync.dma_start(out=kblob_sb[:, 1024:1544],
                              in_=kblob[:, 1024:1544].bitcast(bf16))
            load_u("u2", 4)                              # window 3
            lwt_sb = consts.tile([128, HT, H], bf16, tag="lwt")
            nc.sync.dma_start(out=lwt_sb,
                              in_=lwt[:, :].bitcast(bf16)
                              .rearrange("(a p) o -> p a o", p=128))
            load_u("u", 6)                               # window 4
            load_u("u2", 7)                              # window 5

            y1_sb = consts.tile([128, HT, LLOC], bf16, tag="y1")
            y2_sb = consts.tile([128, HT, LLOC], bf16, tag="y2")

            # ---------- pipeline ----------
            with (
                tc.tile_pool(name="ps_fwd", bufs=2, space="PSUM") as ps_fwd,
                tc.tile_pool(name="ps_y1", bufs=1, space="PSUM") as ps_y1,
                tc.tile_pool(name="ps_lin", bufs=1, space="PSUM") as ps_lin,
            ):
                # activation-table preload in the DMA shadow
                warmact = scratch.tile([128, 1], bf16, tag="wact")
                nc.scalar.activation(out=warmact, in_=warm_sb[:, 0:1], func=GELU)

                # PE clock warmup on a memset tile; values discarded
                wm_ps = ps_y1.tile([128, HT, F], f32, tag="y1ps", name="wm_ps")
                for w in range(NWARM):
                    nc.tensor.matmul(wm_ps[:, 0:2, :].rearrange(
                                         "p a f -> p (a f)")[:, 0:304],
                                     lhsT=warm_sb[:, 0:128],
                                     rhs=warm_sb, start=(w == 0),
                                     stop=(w == NWARM - 1))
                wm_out = scratch.tile([128, 1], bf16, tag="wmout")
                nc.vector.tensor_copy(out=wm_out, in_=wm_ps[:, 0, 0:1])

                fwd_tiles = {}

                def emit_fwd(c):
                    if c % 2 == 0:
                        src, q0 = u_sb, 3 * c // 2
                    else:
                        src, q0 = u2_sb, (3 * c - 1) // 2
                    ur_ps = ps_fwd.tile([128, H], f32, tag="ur", name=f"ur_{c}")
                    ui_ps = ps_fwd.tile([128, H], f32, tag="ui", name=f"ui_{c}")
                    for a in range(2):
                        rhs = src[:, q0 + a, :]
                        nc.tensor.matmul(ur_ps, lhsT=dfc_sb[a], rhs=rhs,
                                         start=(a == 0), stop=(a == 1))
                        nc.tensor.matmul(ui_ps, lhsT=dfsn_sb[a], rhs=rhs,
                                         start=(a == 0), stop=(a == 1))
                    fwd_tiles[c] = (ur_ps, ui_ps)

                def emit_tail(c):
                    ur_ps, ui_ps = fwd_tiles.pop(c)
                    urb = scratch.tile([128, H], bf16, tag="urb", name=f"urb_{c}")
                    m24 = scratch.tile([128, 2, H], bf16, tag="m24",
                                       name=f"m24_{c}")
                    m13 = scratch.tile([128, 2, H], bf16, tag="m13",
                                       name=f"m13_{c}")
                    pq = scratch.tile([128, 2, H], bf16, tag="pq",
                                      name=f"pq_{c}")
                    # fused complex product: ACT casts ur to bf16 (Pool can't
                    # read PSUM); DVE computes both ui products in one wide
                    # mult via a broadcast AP (m24 = [ui*kr | ui*ki]), both
                    # ur products likewise (m13 = [ur*kr | ur*ki]), then
                    # pr = m13.0 - m24.1 and pi = m13.1 + m24.0
                    nc.scalar.activation(out=urb, in_=ur_ps, func=COPY)
                    ur_b = urb.unsqueeze(1).broadcast_to([128, 2, H])
                    pr = pq[:, 0, :]
                    pi = pq[:, 1, :]
                    ui_b = ui_ps.unsqueeze(1).broadcast_to([128, 2, H])
                    nc.vector.tensor_mul(m24, ui_b, kr_pair)
                    if c < NWIN - 2:
                        # Pool computes the pi-lane ur-product mid-stream so
                        # the DVE's pr path (m24, m13.0, pr) finishes early
                        nc.vector.tensor_mul(m13[:, 0, :], urb, krb_sb)
                        nc.gpsimd.tensor_mul(m13[:, 1, :], urb, kib_sb)
                    else:
                        # last window stays all-DVE: a Pool handoff here
                        # delays inv5 and the whole kernel tail
                        nc.vector.tensor_mul(m13, ur_b, kr_pair)
                    nc.vector.tensor_sub(pr, m13[:, 0, :], m24[:, 1, :])
                    nc.vector.tensor_add(pi, m13[:, 1, :], m24[:, 0, :])

                    nt = min(HOP, LLOC - c * HOP)
                    y1_ps = ps_y1.tile([128, HT, F], f32, tag="y1ps",
                                       name=f"y1ps_{c}")
                    for a in range(HT):
                        nc.tensor.matmul(y1_ps[:, a, :nt],
                                         lhsT=pr[:, a * 128:(a + 1) * 128],
                                         rhs=icc_sb[:, HALO:HALO + nt],
                                         start=True, stop=False)
                        nc.tensor.matmul(y1_ps[:, a, :nt],
                                         lhsT=pi[:, a * 128:(a + 1) * 128],
                                         rhs=icsn_sb[:, HALO:HALO + nt],
                                         start=False, stop=True)
                    nc.scalar.activation(out=y1_sb[:, :, c * HOP:c * HOP + nt],
                                         in_=y1_ps[:, :, :nt], func=GELU)

                def emit_lin(c):
                    # linear for window c's time-cols; one combined bias+gelu
                    t0 = c * HOP
                    t1 = min((c + 1) * HOP, LLOC)
                    ntc = t1 - t0
                    if c == NWIN - 1:
                        # final 64-col chunk: all four ao slots fit in one
                        # [128,256] f32 region of a dead fwd bank, so a single
                        # gelu covers them
                        pa = ps_fwd.tile([128, H], f32, tag="ur", name=f"l{c}a")
                        pb = None
                        slots = [pa[:, ao * 64:(ao + 1) * 64] for ao in range(HT)]
                    elif c in (NWIN - 3, NWIN - 2):
                        # penultimate chunk gets its own psum (dead fwd banks)
                        # so it never waits on chunk c-1's gelu read; each
                        # [128,512] f32 tile holds two 256-col ao slots
                        pa = ps_fwd.tile([128, H], f32, tag="ur", name=f"l{c}a")
                        pb = ps_fwd.tile([128, H], f32, tag="ui", name=f"l{c}b")
                        slots = [pa[:, 0:256], pa[:, 256:512],
                                 pb[:, 0:256], pb[:, 256:512]]
                    else:
                        # free dim padded to 256 so each ao's accumulation
                        # region stays within a 2KB PSUM bank
                        y2_ps = ps_lin.tile([128, HT, 256], f32, tag="y2ps",
                                            name=f"y2ps_{c}")
                        slots = [y2_ps[:, ao, :] for ao in range(HT)]
                    for ao in reversed(range(HT)):
                        for ai in range(HT):
                            nc.tensor.matmul(
                                slots[ao][:, :ntc],
                                lhsT=lwt_sb[:, ai, ao * 128:(ao + 1) * 128],
                                rhs=y1_sb[:, ai, t0:t1],
                                start=(ai == 0), stop=(ai == HT - 1))
                    if has_bias:
                        for ao in range(HT):
                            nc.scalar.activation(out=y2_sb[:, ao, t0:t1],
                                                 in_=slots[ao][:, :ntc],
                                                 func=GELU,
                                                 bias=lb_sb[:, ao:ao + 1])
                    elif c == NWIN - 1:
                        nc.scalar.activation(
                            out=y2_sb[:, :, t0:t1],
                            in_=pa[:, 0:HT * 64]
                            .rearrange("p (a t) -> p a t", a=HT),
                            func=GELU)
                    elif c in (NWIN - 3, NWIN - 2):
                        nc.scalar.activation(
                            out=y2_sb[:, 0:2, t0:t1],
                            in_=pa.rearrange("p (a t) -> p a t", a=2)[:, :, :ntc],
                            func=GELU)
                        nc.scalar.activation(
                            out=y2_sb[:, 2:4, t0:t1],
                            in_=pb.rearrange("p (a t) -> p a t", a=2)[:, :, :ntc],
                            func=GELU)
                    else:
                        nc.scalar.activation(out=y2_sb[:, :, t0:t1],
                                             in_=y2_ps[:, :, :ntc], func=GELU)
                    # stores: (0,384) (384,768) (768,960)x2 (960,1024) -- the
                    # penultimate chunk stores per ao-half right behind its
                    # two gelus so those transfers clear the DMA pipe before
                    # the tiny final store
                    if c == 1 or c == NWIN - 1:
                        s0 = t0 if c == NWIN - 1 else 0
                        nc.sync.dma_start(
                            out=y2[:, s0:t1].bitcast(bf16)
                            .rearrange("(a p) t -> p a t", p=128),
                            in_=y2_sb[:, :, s0:t1])
                    elif c in (2, 3):
                        # per-window stores fire right after their own gelu,
                        # keeping the DMA pipe clear for the final store
                        nc.sync.dma_start(
                            out=y2[:, t0:t1].bitcast(bf16)
                            .rearrange("(a p) t -> p a t", p=128),
                            in_=y2_sb[:, :, t0:t1])
                    elif c == NWIN - 2:
                        for half in range(2):
                            nc.sync.dma_start(
                                out=y2[half * 256:(half + 1) * 256, t0:t1]
                                .bitcast(bf16)
                                .rearrange("(a p) t -> p a t", p=128),
                                in_=y2_sb[:, 2 * half:2 * half + 2, t0:t1])

                # software-pipelined emission: window c's forward enters the
                # PE stream before window c-1's chain-dependent inverse; each
                # window's linear chunk follows two windows behind, keeping
                # the PE fed while the next spectrum chain runs
                for c in range(NWIN):
                    emit_fwd(c)
                    if c >= 1:
                        emit_tail(c - 1)
                    if c >= 2:
                        emit_lin(c - 2)
                emit_tail(NWIN - 1)
                emit_lin(NWIN - 2)
                emit_lin(NWIN - 1)

    nc.compile()
    return nc


def _to_bf16_bits(x):
    u = np.ascontiguousarray(x, dtype=np.float32).view(np.uint32)
    r = (u + 0x7FFF + ((u >> 16) & 1)) >> 16
    return r.astype(np.uint16)


def _build_tables(frequencies, decays, W, lin_w, lin_b):
    lam_re = (-np.exp(decays.astype(np.float32))).astype(np.float32)
    m = np.arange(K, dtype=np.float32)
    # match the reference's fp32 rounding of Lam[:,None] * t
    re = (lam_re[:, None] * m[None, :]).astype(np.float32)
    im = (frequencies.astype(np.float32)[:, None] * m[None, :]).astype(np.float32)
    mag = np.exp(re.astype(np.float64))
    sc = mag * np.cos(im.astype(np.float64))          # (N, K) f64
    ssn = -mag * np.sin(im.astype(np.float64))        # (N, K) f64

    fb = np.arange(F // 2, dtype=np.float64) + 0.5
    tt = np.arange(F, dtype=np.float64)
    ang = 2 * np.pi * np.outer(tt, fb) / F
    dfc = np.cos(ang)                                  # (F, F/2) f64
    dfsn = -np.sin(ang)
    iang = 2 * np.pi * np.outer(fb, tt) / F
    icc = (2.0 / F) * np.cos(iang)                     # (F/2, F)
    icsn = (-2.0 / F) * np.sin(iang)

    # time-domain kernel (f64): k[h, m] = W0 @ sc + W1 @ ssn
    W0 = W[..., 0].astype(np.float64)
    W1 = W[..., 1].astype(np.float64)
    k64 = W0 @ sc + W1 @ ssn                           # (H, K)
    # kernel spectrum khat[f, h] = sum_m k[h, m] * e^{-2pi i m (f+1/2)/F}
    khat_r = (k64 @ dfc[:K]).T                         # (F/2, H)
    khat_i = (k64 @ dfsn[:K]).T

    # [dfc_a0 | dfsn_a0 | dfc_a1 | dfsn_a1] -- halves load separately
    dfblob = np.zeros((128, 512), np.uint16)
    dfblob[:, 0:128] = _to_bf16_bits(dfc[0:128])
    dfblob[:, 128:256] = _to_bf16_bits(dfsn[0:128])
    dfblob[:, 256:384] = _to_bf16_bits(dfc[128:256])
    dfblob[:, 384:512] = _to_bf16_bits(dfsn[128:256])

    # fused spectrum-product table: [krb|kib] pair feeds both wide mults
    kblob = np.zeros((128, 1544), np.uint16)
    kblob[:, 0:512] = _to_bf16_bits(khat_r)
    kblob[:, 512:1024] = _to_bf16_bits(khat_i)
    kblob[:, 1024:1280] = _to_bf16_bits(icc)
    kblob[:, 1280:1536] = _to_bf16_bits(icsn)
    kblob[:, 1536:1544] = (
        lin_b.astype(np.float32).reshape(HT, 128).T.copy().view(np.uint16))

    return {
        "dfblob": np.ascontiguousarray(dfblob),
        "kblob": np.ascontiguousarray(kblob),
        "lwt": np.ascontiguousarray(_to_bf16_bits(lin_w.astype(np.float32).T)),
    }


def _make_in_maps(u, tables):
    in_maps = []
    for b in range(B):
        for half in range(2):
            lo = half * LLOC
            uT = np.zeros((RPAD, H), np.uint16)
            a0 = lo - HALO
            s0 = max(a0, 0)
            s1 = min(a0 + ROWS, L)
            uT[s0 - a0:s1 - a0] = _to_bf16_bits(u[b, :, s0:s1].T)
            in_maps.append({"ut": np.ascontiguousarray(uT), **tables})
    return in_maps


def kernel(u, frequencies, decays, W, lin_w, lin_b):
    from concourse.bass_utils import run_bass_kernel_spmd

    u = np.asarray(u, dtype=np.float32)
    tables = _build_tables(np.asarray(frequencies), np.asarray(decays),
                           np.asarray(W), np.asarray(lin_w), np.asarray(lin_b))

    has_bias = bool(np.any(np.asarray(lin_b)))
    key = ("nc", has_bias)
    if key not in _cache:
        _cache[key] = _build_nc(has_bias)
    nc = _cache[key]
    _cache["nc"] = nc

    in_maps = _make_in_maps(u, tables)
    res = run_bass_kernel_spmd(nc, in_maps, core_ids=list(range(NCORES)))
    out = np.empty((B, H, L), np.float32)
    for i, r in enumerate(res.results):
        b, half = divmod(i, 2)
        y = (r["y2"].astype(np.uint32) << 16).view(np.float32)
        out[b, :, half * LLOC:(half + 1) * LLOC] = y
    return out
